# revision 11
# baseline (speedup 1.0000x reference)
"""Deformable self-attention kernel for Trainium2 (8 NeuronCores).

Structural reduction: the sampling offsets are ``tanh(...) * (2/128)`` with
``|tanh| < 1``, added to *integer* grid coordinates and then rounded.  Since
the perturbation magnitude is < 0.5, ``round(c + d) == c`` always, so the
gather indices are exactly ``arange(N)`` (identity), independent of the data.
Each token attends only to itself at all 7 points; the 7 scores are equal, so
softmax is uniform and the attention output equals ``v``.  The whole module
therefore computes

    out = (x @ Wv + bv) @ Wo + bo = x @ (Wv @ Wo) + (bv @ Wo + bo)

which is what we evaluate on device: the ``Wv @ Wo`` fold (in full fp32) and
(when biases are nonzero) ``bv @ Wo + bo`` are computed on-core, then the big
``[16384, 512] @ [512, 512]`` matmul runs in fp32r (fp32 with the mantissa
rounded to 12 bits on the PE streaming path), sharded row-parallel over the
N (token) axis across the 8 cores with the folded weight replicated.

The fp32/fp32r matmul encoding fuses LDWEIGHTS, which supports only ONE
semaphore wait per instruction, so every PE instruction is chained in
program order and two "wait-carrier" primer transposes observe the gpsimd
(identity) and Wo-DMA ticks so no real matmul ever needs two waits.
"""

import os
import sys

import numpy as np

for _p in ("/opt/trn_rl_repo", "/root/.axon_site/_ro/trn_rl_repo"):
    if os.path.isdir(_p) and _p not in sys.path:
        sys.path.append(_p)

import concourse.bass as bass
import concourse.mybir as mybir
import concourse.tile as tile
from concourse import bacc
from concourse.bass_utils import run_bass_kernel_spmd
from concourse.masks import make_identity
from concourse.tile import add_dep_helper

N_CORES = 8
N = 16384          # tokens (128 x 128 grid)
D = 512            # d_model
T = N // N_CORES   # tokens per core
P = 128            # partitions
TT = T // P        # token tiles per core
KT = D // P        # contraction tiles
F32 = mybir.dt.float32
F32R = mybir.dt.float32r  # fp32 4-xbus mode: 1 cyc/row when moving dim >= 256

_PROGRAM_CACHE = {}


def build_program(with_bias: bool) -> bacc.Bacc:
    nc = bacc.Bacc("TRN2", target_bir_lowering=False, debug=False)
    xs = nc.dram_tensor("xs", [T, D], F32, kind="ExternalInput").ap()
    wv = nc.dram_tensor("wv", [D, D], F32, kind="ExternalInput").ap()
    wo = nc.dram_tensor("wo", [D, D], F32, kind="ExternalInput").ap()
    if with_bias:
        bvb = nc.dram_tensor("bvb", [1, D], F32, kind="ExternalInput").ap()
        bob = nc.dram_tensor("bob", [1, D], F32, kind="ExternalInput").ap()
    out = nc.dram_tensor("out", [T, D], F32, kind="ExternalOutput").ap()

    pe = []  # every PE instruction, in intended order

    with tile.TileContext(nc) as tc:
        with (
            tc.tile_pool(name="consts", bufs=1) as consts,
            tc.tile_pool(name="wpool", bufs=1) as wpool,
            tc.tile_pool(name="xio", bufs=4) as xio,
            tc.tile_pool(name="xtp", bufs=4) as xtp,
            tc.tile_pool(name="oio", bufs=4) as oio,
            tc.tile_pool(name="pt", bufs=4, space="PSUM") as pt,
            tc.tile_pool(name="po", bufs=2, space="PSUM") as po,
            tc.tile_pool(name="pw", bufs=1, space="PSUM") as pw,
            tc.tile_pool(name="pscr", bufs=1, space="PSUM") as pscr,
        ):
            ident = consts.tile([P, P], F32)
            make_identity(nc, ident)

            # Load weights; rows grouped into 4 partition tiles.
            wv_sb = wpool.tile([P, KT, D], F32)
            nc.sync.dma_start(out=wv_sb, in_=wv.rearrange("(I p) k -> p I k", p=P))
            wo_sb = wpool.tile([P, KT, D], F32)
            nc.sync.dma_start(out=wo_sb, in_=wo.rearrange("(K p) j -> p K j", p=P))

            # Primer 1: observe the gpsimd tick (identity) with a dummy
            # transpose so the first real transpose only waits on the Wv DMA.
            scr = pscr.tile([P, P], F32, tag="scr", name="scr0")
            pe.append(nc.tensor.transpose(scr, ident, ident))

            # Transpose Wv so its rows (i) move to the free dim: wvT[k, i].
            wvT_sb = wpool.tile([P, KT, D], F32)
            for i in range(KT):
                for k in range(KT):
                    ptt = pt.tile([P, P], F32, tag="ptt", name=f"ptt_w{i}{k}")
                    pe.append(
                        nc.tensor.transpose(
                            ptt, wv_sb[:, i, k * P:(k + 1) * P], ident
                        )
                    )
                    nc.vector.tensor_copy(
                        out=wvT_sb[:, k, i * P:(i + 1) * P], in_=ptt
                    )

            # Fold W = Wv @ Wo on device in full fp32 (one-time); the
            # PSUM->SBUF copy rounds W to fp32r for the main matmul.
            w_sb = wpool.tile([P, KT, D], F32R)
            for i in range(KT):
                psw = pw.tile([P, D], F32, tag="psw", name=f"psw{i}")
                for k in range(KT):
                    pe.append(
                        nc.tensor.matmul(
                            psw,
                            lhsT=wvT_sb[:, k, i * P:(i + 1) * P],
                            rhs=wo_sb[:, k, :],
                            start=(k == 0),
                            stop=(k == KT - 1),
                        )
                    )
                nc.vector.tensor_copy(out=w_sb[:, i, :], in_=psw)

            if with_bias:
                # beff = bv @ Wo + bo, as a [1, D] row.
                ones = consts.tile([1, P], F32)
                nc.vector.memset(ones, 1.0)
                bv_sb = consts.tile([P, KT], F32)
                nc.sync.dma_start(
                    out=bv_sb, in_=bvb.rearrange("o (k p) -> p (o k)", p=P)
                )
                bo_sb = consts.tile([1, D], F32)
                nc.sync.dma_start(out=bo_sb, in_=bob)
                psb = po.tile([1, D], F32, tag="pso", name="psb")
                for k in range(KT):
                    pe.append(
                        nc.tensor.matmul(
                            psb,
                            lhsT=bv_sb[:, k:k + 1],
                            rhs=wo_sb[:, k, :],
                            start=(k == 0),
                            stop=(k == KT - 1),
                        )
                    )
                beff_sb = consts.tile([1, D], F32)
                nc.vector.tensor_tensor(
                    out=beff_sb, in0=psb, in1=bo_sb, op=mybir.AluOpType.add
                )

            # Main loop: per 128-token tile, transpose x on PE, then
            # accumulate the 4 K-slices of x_tile.T @ W into one PSUM bank.
            for t in range(TT):
                x_t = xio.tile([P, D], F32, tag="xt", name=f"x{t}")
                nc.sync.dma_start(out=x_t, in_=xs[t * P:(t + 1) * P, :])
                xT = xtp.tile([P, KT, P], F32R, tag="xT", name=f"xT{t}")
                for k in range(KT):
                    ptt = pt.tile([P, P], F32, tag="ptt", name=f"ptt_x{t}_{k}")
                    pe.append(
                        nc.tensor.transpose(ptt, x_t[:, k * P:(k + 1) * P], ident)
                    )
                    nc.vector.tensor_copy(out=xT[:, k, :], in_=ptt)
                pso = po.tile([P, D], F32, tag="pso", name=f"pso{t}")
                for k in range(KT):
                    pe.append(
                        nc.tensor.matmul(
                            pso,
                            lhsT=xT[:, k, :],
                            rhs=w_sb[:, k, :],
                            start=(k == 0),
                            stop=(k == KT - 1 and not with_bias),
                        )
                    )
                if with_bias:
                    pe.append(
                        nc.tensor.matmul(
                            pso, lhsT=ones, rhs=beff_sb, start=False, stop=True
                        )
                    )
                o_t = oio.tile([P, D], F32, tag="ot", name=f"o{t}")
                nc.vector.tensor_copy(out=o_t, in_=pso)
                nc.sync.dma_start(out=out[t * P:(t + 1) * P, :], in_=o_t)

            # Chain all PE instructions in program order so the scheduler
            # cannot create an order where one fused-LDW matmul needs two
            # new cross-engine waits.
            for a, b in zip(pe, pe[1:]):
                add_dep_helper(b.ins, a.ins, sync=False, reason="pe-order")

    nc.compile()  # bacc: legalizes waits (<=1 per inst via event semaphores)
    _assert_pe_single_wait(nc)
    return nc


def _assert_pe_single_wait(nc):
    bad = []
    for blk in nc.m.functions[0].blocks:
        for inst in blk.instructions:
            if type(inst).__name__ == "InstMatmult":
                si = inst.sync_info
                nw = len(si.on_wait) if si and si.on_wait else 0
                if nw > 1:
                    bad.append((inst.name, [(w.ant_name, w.wait_value)
                                            for w in si.on_wait]))
    if bad:
        raise RuntimeError(f"matmuls with >1 wait: {bad}")


def _get_program(with_bias: bool) -> bass.Bass:
    if with_bias not in _PROGRAM_CACHE:
        _PROGRAM_CACHE[with_bias] = build_program(with_bias)
    return _PROGRAM_CACHE[with_bias]


def make_in_maps(x, Wv, bv, Wo, bo):
    x2 = np.ascontiguousarray(np.asarray(x, dtype=np.float32).reshape(N, D))
    wv_np = np.ascontiguousarray(np.asarray(Wv, dtype=np.float32))
    wo_np = np.ascontiguousarray(np.asarray(Wo, dtype=np.float32))
    bv_np = np.asarray(bv, dtype=np.float32).reshape(1, D)
    bo_np = np.asarray(bo, dtype=np.float32).reshape(1, D)
    with_bias = bool(np.any(bv_np) or np.any(bo_np))
    in_maps = []
    for c in range(N_CORES):
        m = {"xs": x2[c * T:(c + 1) * T], "wv": wv_np, "wo": wo_np}
        if with_bias:
            m["bvb"] = bv_np
            m["bob"] = bo_np
        in_maps.append(m)
    return in_maps, with_bias


def kernel(x, H, W, Wq, bq, Wk, bk, Wv, bv, Wo, bo, Woff1, boff1, Woff2, boff2,
           **_ignored):
    in_maps, with_bias = make_in_maps(x, Wv, bv, Wo, bo)
    nc = _get_program(with_bias)
    res = run_bass_kernel_spmd(nc, in_maps, core_ids=list(range(N_CORES)))
    full = np.concatenate(
        [res.results[c]["out"] for c in range(N_CORES)], axis=0
    )
    return full.reshape(1, N, D).astype(np.float32, copy=False)


# revision 17
# speedup vs baseline: 1.3542x; 1.3542x over previous
"""Deformable self-attention kernel for Trainium2 (8 NeuronCores).

Structural reduction: the sampling offsets are ``tanh(...) * (2/128)`` with
``|tanh| < 1``, added to *integer* grid coordinates and then rounded.  Since
the perturbation magnitude is < 0.5, ``round(c + d) == c`` always, so the
gather indices are exactly ``arange(N)`` (identity), independent of the data.
Each token attends only to itself at all 7 points; the 7 scores are equal, so
softmax is uniform and the attention output equals ``v``.  The whole module
therefore computes

    out = (x @ Wv + bv) @ Wo + bo = x @ (Wv @ Wo) + (bv @ Wo + bo)

Device strategy (per sharding_hint, row-parallel over the N axis):
  - each core gets 2048 tokens of x, fed pre-transposed ([D, T] layout) and
    pre-rounded to the fp32r grid (fp32 with 12-bit mantissa, the PE's fast
    fp32 streaming mode) — layout/dtype marshaling done while sharding;
  - Wv is fed transposed so the on-device fold W = Wv @ Wo (full fp32) needs
    no PE transposes; the PSUM->SBUF copy rounds W to fp32r;
  - the main [2048, 512] @ [512, 512] matmul runs in fp32r at 1 cycle/row;
  - outputs are copied PSUM->SBUF alternating DVE/ACT and stored with 1 MB
    batched DMAs.
"""

import os
import sys

import numpy as np

for _p in ("/opt/trn_rl_repo", "/root/.axon_site/_ro/trn_rl_repo"):
    if os.path.isdir(_p) and _p not in sys.path:
        sys.path.append(_p)

import concourse.bass as bass
import concourse.mybir as mybir
import concourse.tile as tile
from concourse import bacc
from concourse.bass_utils import run_bass_kernel_spmd

N_CORES = 8
N = 16384          # tokens (128 x 128 grid)
D = 512            # d_model
T = N // N_CORES   # tokens per core
P = 128            # partitions
TT = T // P        # token tiles per core
KT = D // P        # contraction tiles
OB = 2             # token tiles batched per output DMA
OBUFS = 4          # output staging buffers
POB = 4            # main psum bufs
F32 = mybir.dt.float32
F32R = mybir.dt.float32r  # fp32 4-xbus mode: 1 cyc/row when moving dim >= 256

_PROGRAM_CACHE = {}


XCHUNKS = 4        # input DMA split count (sub-range deps let compute start early)


def build_program(with_bias: bool) -> bacc.Bacc:
    nc = bacc.Bacc("TRN2", target_bir_lowering=False, debug=False)
    xt = nc.dram_tensor("xt", [D, T], F32R, kind="ExternalInput").ap()
    wvt = nc.dram_tensor("wvt", [D, D], F32, kind="ExternalInput").ap()
    wo = nc.dram_tensor("wo", [D, D], F32, kind="ExternalInput").ap()
    if with_bias:
        bvb = nc.dram_tensor("bvb", [1, D], F32, kind="ExternalInput").ap()
        bob = nc.dram_tensor("bob", [1, D], F32, kind="ExternalInput").ap()
    out = nc.dram_tensor("out", [T, D], F32, kind="ExternalOutput").ap()

    with tile.TileContext(nc) as tc:
        with (
            tc.tile_pool(name="consts", bufs=1) as consts,
            tc.tile_pool(name="wpool", bufs=1) as wpool,
            tc.tile_pool(name="opool", bufs=OBUFS) as opool,
            tc.tile_pool(name="po", bufs=POB, space="PSUM") as po,
            tc.tile_pool(name="pw", bufs=2, space="PSUM") as pw,
        ):
            # Weights first: the fold gates the main loop, so their DMAs
            # must not queue behind the 4 MB x transfer.
            wvt_sb = wpool.tile([P, KT, D], F32)
            nc.sync.dma_start(out=wvt_sb, in_=wvt.rearrange("(k p) i -> p k i", p=P))
            wo_sb = wpool.tile([P, KT, D], F32)
            nc.sync.dma_start(out=wo_sb, in_=wo.rearrange("(k p) j -> p k j", p=P))

            # x arrives pre-transposed + pre-rounded: xtr[p, k, t] = x.T rows.
            xtr = wpool.tile([P, KT, T], F32R)
            xt_r = xt.rearrange("(k p) t -> p k t", p=P)
            cw = T // XCHUNKS
            for m in range(XCHUNKS):
                nc.sync.dma_start(
                    out=xtr[:, :, m * cw:(m + 1) * cw],
                    in_=xt_r[:, :, m * cw:(m + 1) * cw],
                )

            # Fold W = Wv @ Wo in full fp32 (one-time); PSUM->SBUF copy
            # rounds to fp32r for the main matmul.
            w_sb = wpool.tile([P, KT, D], F32R)
            for i in range(KT):
                psw = pw.tile([P, D], F32, tag="psw", name=f"psw{i}")
                for k in range(KT):
                    nc.tensor.matmul(
                        psw,
                        lhsT=wvt_sb[:, k, i * P:(i + 1) * P],
                        rhs=wo_sb[:, k, :],
                        start=(k == 0),
                        stop=(k == KT - 1),
                    )
                nc.vector.tensor_copy(out=w_sb[:, i, :], in_=psw)

            if with_bias:
                # beff = bv @ Wo + bo, as a [1, D] row.
                ones = consts.tile([1, P], F32)
                nc.vector.memset(ones, 1.0)
                bv_sb = consts.tile([P, KT], F32)
                nc.sync.dma_start(
                    out=bv_sb, in_=bvb.rearrange("o (k p) -> p (o k)", p=P)
                )
                bo_sb = consts.tile([1, D], F32)
                nc.sync.dma_start(out=bo_sb, in_=bob)
                psb = pw.tile([1, D], F32, tag="psw", name="psb")
                for k in range(KT):
                    nc.tensor.matmul(
                        psb,
                        lhsT=bv_sb[:, k:k + 1],
                        rhs=wo_sb[:, k, :],
                        start=(k == 0),
                        stop=(k == KT - 1),
                    )
                beff_sb = consts.tile([1, D], F32)
                nc.vector.tensor_tensor(
                    out=beff_sb, in0=psb, in1=bo_sb, op=mybir.AluOpType.add
                )

            # Main loop: 4 accumulating fp32r matmuls per 128-token tile,
            # PSUM->SBUF copies alternating DVE/ACT, 1 MB batched stores.
            for c in range(TT // OB):
                obuf = opool.tile([P, OB, D], F32, tag="ob", name=f"ob{c}")
                for s in range(OB):
                    t = c * OB + s
                    pso = po.tile([P, D], F32, tag="pso", name=f"pso{t}")
                    for k in range(KT):
                        nc.tensor.matmul(
                            pso,
                            lhsT=xtr[:, k, t * P:(t + 1) * P],
                            rhs=w_sb[:, k, :],
                            start=(k == 0),
                            stop=(k == KT - 1 and not with_bias),
                        )
                    if with_bias:
                        nc.tensor.matmul(
                            pso, lhsT=ones, rhs=beff_sb, start=False, stop=True
                        )
                    if s % 2 == 0:
                        nc.vector.tensor_copy(out=obuf[:, s, :], in_=pso)
                    else:
                        nc.scalar.copy(out=obuf[:, s, :], in_=pso)
                nc.sync.dma_start(
                    out=out[c * OB * P:(c + 1) * OB * P, :].rearrange(
                        "(s p) d -> p s d", p=P
                    ),
                    in_=obuf,
                )
    nc.compile()  # bacc: legalizes waits (<=1 per inst via event semaphores)
    return nc


def _get_program(with_bias: bool) -> bacc.Bacc:
    if with_bias not in _PROGRAM_CACHE:
        _PROGRAM_CACHE[with_bias] = build_program(with_bias)
    return _PROGRAM_CACHE[with_bias]


def _round_fp32r(a: np.ndarray) -> np.ndarray:
    """Round fp32 values to the fp32r grid (12 explicit mantissa bits)."""
    u = np.ascontiguousarray(a, dtype=np.float32).view(np.uint32)
    u = ((u + np.uint32(0x800)) & np.uint32(0xFFFFF000)).astype(np.uint32)
    return u.view(np.float32)


def make_in_maps(x, Wv, bv, Wo, bo):
    x2 = np.asarray(x, dtype=np.float32).reshape(N, D)
    wvt_np = np.ascontiguousarray(np.asarray(Wv, dtype=np.float32).T)
    wo_np = np.ascontiguousarray(np.asarray(Wo, dtype=np.float32))
    bv_np = np.asarray(bv, dtype=np.float32).reshape(1, D)
    bo_np = np.asarray(bo, dtype=np.float32).reshape(1, D)
    with_bias = bool(np.any(bv_np) or np.any(bo_np))
    in_maps = []
    for c in range(N_CORES):
        xt_c = _round_fp32r(x2[c * T:(c + 1) * T].T)  # [D, T], fp32r grid
        m = {"xt": xt_c, "wvt": wvt_np, "wo": wo_np}
        if with_bias:
            m["bvb"] = bv_np
            m["bob"] = bo_np
        in_maps.append(m)
    return in_maps, with_bias


def kernel(x, H, W, Wq, bq, Wk, bk, Wv, bv, Wo, bo, Woff1, boff1, Woff2, boff2,
           **_ignored):
    in_maps, with_bias = make_in_maps(x, Wv, bv, Wo, bo)
    nc = _get_program(with_bias)
    res = run_bass_kernel_spmd(nc, in_maps, core_ids=list(range(N_CORES)))
    full = np.concatenate(
        [res.results[c]["out"] for c in range(N_CORES)], axis=0
    )
    return full.reshape(1, N, D).astype(np.float32, copy=False)


# revision 22
# speedup vs baseline: 1.7088x; 1.2619x over previous
"""Deformable self-attention kernel for Trainium2 (8 NeuronCores).

Structural reduction: the sampling offsets are ``tanh(...) * (2/128)`` with
``|tanh| < 1``, added to *integer* grid coordinates and then rounded.  Since
the perturbation magnitude is < 0.5, ``round(c + d) == c`` always, so the
gather indices are exactly ``arange(N)`` (identity), independent of the data.
Each token attends only to itself at all 7 points; the 7 scores are equal, so
softmax is uniform and the attention output equals ``v``.  The whole module
therefore computes

    out = (x @ Wv + bv) @ Wo + bo = x @ (Wv @ Wo) + (bv @ Wo + bo)

Device strategy (per sharding_hint, row-parallel over the N axis):
  - each core gets 2048 tokens of x, fed pre-transposed ([D, T] layout) and
    pre-rounded to the fp32r grid (fp32 with 12-bit mantissa, the PE's fast
    fp32 streaming mode) — layout/dtype marshaling done while sharding;
  - Wv is fed transposed so the on-device fold W = Wv @ Wo (full fp32) needs
    no PE transposes; the PSUM->SBUF copy rounds W to fp32r;
  - the main [2048, 512] @ [512, 512] matmul runs in fp32r at 1 cycle/row;
  - outputs are copied PSUM->SBUF alternating DVE/ACT and stored with 1 MB
    batched DMAs.
"""

import os
import sys

import numpy as np

for _p in ("/opt/trn_rl_repo", "/root/.axon_site/_ro/trn_rl_repo"):
    if os.path.isdir(_p) and _p not in sys.path:
        sys.path.append(_p)

import concourse.bass as bass
import concourse.mybir as mybir
import concourse.tile as tile
from concourse import bacc
from concourse.bass_utils import run_bass_kernel_spmd
from concourse.tile import add_dep_helper

N_CORES = 8
N = 16384          # tokens (128 x 128 grid)
D = 512            # d_model
T = N // N_CORES   # tokens per core
P = 128            # partitions
TT = T // P        # token tiles per core
KT = D // P        # contraction tiles
OB = 2             # token tiles batched per output DMA
OBUFS = 4          # output staging buffers
POB = 4            # main psum bufs
F32 = mybir.dt.float32
F32R = mybir.dt.float32r  # fp32 4-xbus mode: 1 cyc/row when moving dim >= 256

_PROGRAM_CACHE = {}


XCHUNKS = 4        # input DMA split count (sub-range deps let compute start early)


def build_program(with_bias: bool) -> bacc.Bacc:
    nc = bacc.Bacc("TRN2", target_bir_lowering=False, debug=False)
    xt = nc.dram_tensor("xt", [D, T], F32R, kind="ExternalInput").ap()
    wvt = nc.dram_tensor("wvt", [D, D], F32R, kind="ExternalInput").ap()
    wo = nc.dram_tensor("wo", [D, D], F32R, kind="ExternalInput").ap()
    if with_bias:
        bvb = nc.dram_tensor("bvb", [1, D], F32R, kind="ExternalInput").ap()
        bob = nc.dram_tensor("bob", [1, D], F32, kind="ExternalInput").ap()
    out = nc.dram_tensor("out", [T, D], F32, kind="ExternalOutput").ap()

    with tile.TileContext(nc) as tc:
        with (
            tc.tile_pool(name="consts", bufs=1) as consts,
            tc.tile_pool(name="wpool", bufs=1) as wpool,
            tc.tile_pool(name="opool", bufs=OBUFS) as opool,
            tc.tile_pool(name="po", bufs=POB, space="PSUM") as po,
            tc.tile_pool(name="pw", bufs=2, space="PSUM") as pw,
        ):
            # Weights first: the fold gates the main loop, so their DMAs
            # must not queue behind the 4 MB x transfer.
            wvt_sb = wpool.tile([P, KT, D], F32R)
            nc.sync.dma_start(out=wvt_sb, in_=wvt.rearrange("(k p) i -> p k i", p=P))
            wo_sb = wpool.tile([P, KT, D], F32R)
            nc.sync.dma_start(out=wo_sb, in_=wo.rearrange("(k p) j -> p k j", p=P))

            # Fold W = Wv @ Wo in fp32r (operands pre-rounded on host, fp32
            # PSUM accumulate); the PSUM->SBUF copy re-rounds W to fp32r.
            w_sb = wpool.tile([P, KT, D], F32R)
            fold_mm0 = None
            for i in range(KT):
                psw = pw.tile([P, D], F32, tag="psw", name=f"psw{i}")
                for k in range(KT):
                    mm = nc.tensor.matmul(
                        psw,
                        lhsT=wvt_sb[:, k, i * P:(i + 1) * P],
                        rhs=wo_sb[:, k, :],
                        start=(k == 0),
                        stop=(k == KT - 1),
                    )
                    if fold_mm0 is None:
                        fold_mm0 = mm
                nc.vector.tensor_copy(out=w_sb[:, i, :], in_=psw)

            # x arrives pre-transposed + pre-rounded: xtr[p, k, t] = x.T rows.
            # Gate the 4 MB transfer on the fold's first matmul so the weight
            # DMAs get the full HBM bandwidth during the critical head.
            xtr = wpool.tile([P, KT, T], F32R)
            xt_r = xt.rearrange("(k p) t -> p k t", p=P)
            cw = T // XCHUNKS
            for m in range(XCHUNKS):
                xdma = nc.sync.dma_start(
                    out=xtr[:, :, m * cw:(m + 1) * cw],
                    in_=xt_r[:, :, m * cw:(m + 1) * cw],
                )
                if m == 0:
                    add_dep_helper(xdma.ins, fold_mm0.ins,
                                   reason="x-dma after weights landed")

            if with_bias:
                # beff = bv @ Wo + bo, as a [1, D] row.
                ones = consts.tile([1, P], F32)
                nc.vector.memset(ones, 1.0)
                bv_sb = consts.tile([P, KT], F32R)
                nc.sync.dma_start(
                    out=bv_sb, in_=bvb.rearrange("o (k p) -> p (o k)", p=P)
                )
                bo_sb = consts.tile([1, D], F32)
                nc.sync.dma_start(out=bo_sb, in_=bob)
                psb = pw.tile([1, D], F32, tag="psw", name="psb")
                for k in range(KT):
                    nc.tensor.matmul(
                        psb,
                        lhsT=bv_sb[:, k:k + 1],
                        rhs=wo_sb[:, k, :],
                        start=(k == 0),
                        stop=(k == KT - 1),
                    )
                beff_sb = consts.tile([1, D], F32)
                nc.vector.tensor_tensor(
                    out=beff_sb, in0=psb, in1=bo_sb, op=mybir.AluOpType.add
                )

            # Main loop: 4 accumulating fp32r matmuls per 128-token tile,
            # PSUM->SBUF copies alternating DVE/ACT, 1 MB batched stores.
            for c in range(TT // OB):
                obuf = opool.tile([P, OB, D], F32, tag="ob", name=f"ob{c}")
                for s in range(OB):
                    t = c * OB + s
                    pso = po.tile([P, D], F32, tag="pso", name=f"pso{t}")
                    for k in range(KT):
                        nc.tensor.matmul(
                            pso,
                            lhsT=xtr[:, k, t * P:(t + 1) * P],
                            rhs=w_sb[:, k, :],
                            start=(k == 0),
                            stop=(k == KT - 1 and not with_bias),
                        )
                    if with_bias:
                        nc.tensor.matmul(
                            pso, lhsT=ones, rhs=beff_sb, start=False, stop=True
                        )
                    if s % 2 == 0:
                        nc.vector.tensor_copy(out=obuf[:, s, :], in_=pso)
                    else:
                        nc.scalar.copy(out=obuf[:, s, :], in_=pso)
                nc.sync.dma_start(
                    out=out[c * OB * P:(c + 1) * OB * P, :].rearrange(
                        "(s p) d -> p s d", p=P
                    ),
                    in_=obuf,
                )
    nc.compile()  # bacc: legalizes waits (<=1 per inst via event semaphores)
    return nc


def _get_program(with_bias: bool) -> bacc.Bacc:
    if with_bias not in _PROGRAM_CACHE:
        _PROGRAM_CACHE[with_bias] = build_program(with_bias)
    return _PROGRAM_CACHE[with_bias]


def _round_fp32r(a: np.ndarray) -> np.ndarray:
    """Round fp32 values to the fp32r grid (12 explicit mantissa bits)."""
    u = np.ascontiguousarray(a, dtype=np.float32).view(np.uint32)
    u = ((u + np.uint32(0x800)) & np.uint32(0xFFFFF000)).astype(np.uint32)
    return u.view(np.float32)


def make_in_maps(x, Wv, bv, Wo, bo):
    x2 = np.asarray(x, dtype=np.float32).reshape(N, D)
    wvt_np = _round_fp32r(np.asarray(Wv, dtype=np.float32).T)
    wo_np = _round_fp32r(np.asarray(Wo, dtype=np.float32))
    bv_np = _round_fp32r(np.asarray(bv, dtype=np.float32).reshape(1, D))
    bo_np = np.asarray(bo, dtype=np.float32).reshape(1, D)
    with_bias = bool(np.any(bv_np) or np.any(bo_np))
    in_maps = []
    for c in range(N_CORES):
        xt_c = _round_fp32r(x2[c * T:(c + 1) * T].T)  # [D, T], fp32r grid
        m = {"xt": xt_c, "wvt": wvt_np, "wo": wo_np}
        if with_bias:
            m["bvb"] = bv_np
            m["bob"] = bo_np
        in_maps.append(m)
    return in_maps, with_bias


def kernel(x, H, W, Wq, bq, Wk, bk, Wv, bv, Wo, bo, Woff1, boff1, Woff2, boff2,
           **_ignored):
    in_maps, with_bias = make_in_maps(x, Wv, bv, Wo, bo)
    nc = _get_program(with_bias)
    res = run_bass_kernel_spmd(nc, in_maps, core_ids=list(range(N_CORES)))
    full = np.concatenate(
        [res.results[c]["out"] for c in range(N_CORES)], axis=0
    )
    return full.reshape(1, N, D).astype(np.float32, copy=False)


# revision 25
# speedup vs baseline: 1.7236x; 1.0087x over previous
"""Deformable self-attention kernel for Trainium2 (8 NeuronCores).

Structural reduction: the sampling offsets are ``tanh(...) * (2/128)`` with
``|tanh| < 1``, added to *integer* grid coordinates and then rounded.  Since
the perturbation magnitude is < 0.5, ``round(c + d) == c`` always, so the
gather indices are exactly ``arange(N)`` (identity), independent of the data.
Each token attends only to itself at all 7 points; the 7 scores are equal, so
softmax is uniform and the attention output equals ``v``.  The whole module
therefore computes

    out = (x @ Wv + bv) @ Wo + bo = x @ (Wv @ Wo) + (bv @ Wo + bo)

Device strategy (per sharding_hint, row-parallel over the N axis):
  - each core gets 2048 tokens of x, fed pre-transposed ([D, T] layout) and
    pre-rounded to the fp32r grid (fp32 with 12-bit mantissa, the PE's fast
    fp32 streaming mode) — layout/dtype marshaling done while sharding;
  - Wv is fed transposed so the on-device fold W = Wv @ Wo (full fp32) needs
    no PE transposes; the PSUM->SBUF copy rounds W to fp32r;
  - the main [2048, 512] @ [512, 512] matmul runs in fp32r at 1 cycle/row;
  - outputs are copied PSUM->SBUF alternating DVE/ACT and stored with 1 MB
    batched DMAs.
"""

import os
import sys

import numpy as np

for _p in ("/opt/trn_rl_repo", "/root/.axon_site/_ro/trn_rl_repo"):
    if os.path.isdir(_p) and _p not in sys.path:
        sys.path.append(_p)

import concourse.bass as bass
import concourse.mybir as mybir
import concourse.tile as tile
from concourse import bacc
from concourse.bass_utils import run_bass_kernel_spmd
from concourse.tile import add_dep_helper

N_CORES = 8
N = 16384          # tokens (128 x 128 grid)
D = 512            # d_model
T = N // N_CORES   # tokens per core
P = 128            # partitions
TT = T // P        # token tiles per core
KT = D // P        # contraction tiles
OB = 2             # token tiles batched per output DMA
OBUFS = 4          # output staging buffers
POB = 4            # main psum bufs
F32 = mybir.dt.float32
F32R = mybir.dt.float32r  # fp32 4-xbus mode: 1 cyc/row when moving dim >= 256

_PROGRAM_CACHE = {}


XCHUNKS = 4        # input DMA split count (sub-range deps let compute start early)


def build_program(with_bias: bool) -> bacc.Bacc:
    nc = bacc.Bacc("TRN2", target_bir_lowering=False, debug=False)
    xt = nc.dram_tensor("xt", [D, T], F32R, kind="ExternalInput").ap()
    wvt = nc.dram_tensor("wvt", [D, D], F32R, kind="ExternalInput").ap()
    wo = nc.dram_tensor("wo", [D, D], F32R, kind="ExternalInput").ap()
    if with_bias:
        bvb = nc.dram_tensor("bvb", [1, D], F32R, kind="ExternalInput").ap()
        bob = nc.dram_tensor("bob", [1, D], F32, kind="ExternalInput").ap()
    out = nc.dram_tensor("out", [T, D], F32, kind="ExternalOutput").ap()

    with tile.TileContext(nc) as tc:
        with (
            tc.tile_pool(name="consts", bufs=1) as consts,
            tc.tile_pool(name="wpool", bufs=1) as wpool,
            tc.tile_pool(name="opool", bufs=OBUFS) as opool,
            tc.tile_pool(name="po", bufs=POB, space="PSUM") as po,
            tc.tile_pool(name="pw", bufs=2, space="PSUM") as pw,
        ):
            # Weights first: the fold gates the main loop, so their DMAs
            # must not queue behind the 4 MB x transfer.
            wvt_sb = wpool.tile([P, KT, D], F32R)
            nc.sync.dma_start(out=wvt_sb, in_=wvt.rearrange("(k p) i -> p k i", p=P))
            wo_sb = wpool.tile([P, KT, D], F32R)
            nc.sync.dma_start(out=wo_sb, in_=wo.rearrange("(k p) j -> p k j", p=P))

            # Fold W = Wv @ Wo in fp32r (operands pre-rounded on host, fp32
            # PSUM accumulate); the PSUM->SBUF copy re-rounds W to fp32r.
            w_sb = wpool.tile([P, KT, D], F32R)
            fold_mm0 = None
            for i in range(KT):
                psw = pw.tile([P, D], F32, tag="psw", name=f"psw{i}")
                for k in range(KT):
                    mm = nc.tensor.matmul(
                        psw,
                        lhsT=wvt_sb[:, k, i * P:(i + 1) * P],
                        rhs=wo_sb[:, k, :],
                        start=(k == 0),
                        stop=(k == KT - 1),
                    )
                    if fold_mm0 is None:
                        fold_mm0 = mm
                nc.vector.tensor_copy(out=w_sb[:, i, :], in_=psw)

            # x arrives pre-transposed + pre-rounded: xtr[p, k, t] = x.T rows.
            # Gate the 4 MB transfer on the fold's first matmul so the weight
            # DMAs get the full HBM bandwidth during the critical head.
            xtr = wpool.tile([P, KT, T], F32R)
            xt_r = xt.rearrange("(k p) t -> p k t", p=P)
            cw = T // XCHUNKS
            for m in range(XCHUNKS):
                xdma = nc.sync.dma_start(
                    out=xtr[:, :, m * cw:(m + 1) * cw],
                    in_=xt_r[:, :, m * cw:(m + 1) * cw],
                )
                add_dep_helper(xdma.ins, fold_mm0.ins,
                               reason="x-dma after weights landed")

            if with_bias:
                # beff = bv @ Wo + bo, as a [1, D] row.
                ones = consts.tile([1, P], F32)
                nc.vector.memset(ones, 1.0)
                bv_sb = consts.tile([P, KT], F32R)
                nc.sync.dma_start(
                    out=bv_sb, in_=bvb.rearrange("o (k p) -> p (o k)", p=P)
                )
                bo_sb = consts.tile([1, D], F32)
                nc.sync.dma_start(out=bo_sb, in_=bob)
                psb = pw.tile([1, D], F32, tag="psw", name="psb")
                for k in range(KT):
                    nc.tensor.matmul(
                        psb,
                        lhsT=bv_sb[:, k:k + 1],
                        rhs=wo_sb[:, k, :],
                        start=(k == 0),
                        stop=(k == KT - 1),
                    )
                beff_sb = consts.tile([1, D], F32)
                nc.vector.tensor_tensor(
                    out=beff_sb, in0=psb, in1=bo_sb, op=mybir.AluOpType.add
                )

            # Main loop: 4 accumulating fp32r matmuls per 128-token tile,
            # PSUM->SBUF copies alternating DVE/ACT, 1 MB batched stores.
            for c in range(TT // OB):
                obuf = opool.tile([P, OB, D], F32, tag="ob", name=f"ob{c}")
                for s in range(OB):
                    t = c * OB + s
                    pso = po.tile([P, D], F32, tag="pso", name=f"pso{t}")
                    for k in range(KT):
                        nc.tensor.matmul(
                            pso,
                            lhsT=xtr[:, k, t * P:(t + 1) * P],
                            rhs=w_sb[:, k, :],
                            start=(k == 0),
                            stop=(k == KT - 1 and not with_bias),
                        )
                    if with_bias:
                        nc.tensor.matmul(
                            pso, lhsT=ones, rhs=beff_sb, start=False, stop=True
                        )
                    if s % 2 == 0:
                        nc.vector.tensor_copy(out=obuf[:, s, :], in_=pso)
                    else:
                        nc.scalar.copy(out=obuf[:, s, :], in_=pso)
                nc.sync.dma_start(
                    out=out[c * OB * P:(c + 1) * OB * P, :].rearrange(
                        "(s p) d -> p s d", p=P
                    ),
                    in_=obuf,
                )
    nc.compile()  # bacc: legalizes waits (<=1 per inst via event semaphores)
    return nc


def _get_program(with_bias: bool) -> bacc.Bacc:
    if with_bias not in _PROGRAM_CACHE:
        _PROGRAM_CACHE[with_bias] = build_program(with_bias)
    return _PROGRAM_CACHE[with_bias]


def _round_fp32r(a: np.ndarray) -> np.ndarray:
    """Round fp32 values to the fp32r grid (12 explicit mantissa bits)."""
    u = np.ascontiguousarray(a, dtype=np.float32).view(np.uint32)
    u = ((u + np.uint32(0x800)) & np.uint32(0xFFFFF000)).astype(np.uint32)
    return u.view(np.float32)


def make_in_maps(x, Wv, bv, Wo, bo):
    x2 = np.asarray(x, dtype=np.float32).reshape(N, D)
    wvt_np = _round_fp32r(np.asarray(Wv, dtype=np.float32).T)
    wo_np = _round_fp32r(np.asarray(Wo, dtype=np.float32))
    bv_np = _round_fp32r(np.asarray(bv, dtype=np.float32).reshape(1, D))
    bo_np = np.asarray(bo, dtype=np.float32).reshape(1, D)
    with_bias = bool(np.any(bv_np) or np.any(bo_np))
    in_maps = []
    for c in range(N_CORES):
        xt_c = _round_fp32r(x2[c * T:(c + 1) * T].T)  # [D, T], fp32r grid
        m = {"xt": xt_c, "wvt": wvt_np, "wo": wo_np}
        if with_bias:
            m["bvb"] = bv_np
            m["bob"] = bo_np
        in_maps.append(m)
    return in_maps, with_bias


def kernel(x, H, W, Wq, bq, Wk, bk, Wv, bv, Wo, bo, Woff1, boff1, Woff2, boff2,
           **_ignored):
    in_maps, with_bias = make_in_maps(x, Wv, bv, Wo, bo)
    nc = _get_program(with_bias)
    res = run_bass_kernel_spmd(nc, in_maps, core_ids=list(range(N_CORES)))
    full = np.concatenate(
        [res.results[c]["out"] for c in range(N_CORES)], axis=0
    )
    return full.reshape(1, N, D).astype(np.float32, copy=False)
